# revision 1
# baseline (speedup 1.0000x reference)
"""Trainium2 Bass kernel for nn_AttentionHead (B=8, N=2048, D=512, d=64).

Reference semantics (faithful to the torch original):
    K = key_input   @ W_key        # note: W_key used for Q, K AND V
    Q = query_input @ W_key
    V = value_input @ W_key
    S = Q @ K^T / sqrt(512)        # scaled by INPUT dim, not head dim
    S = mask(padding), causal-mask if masked_attention
    out = softmax(S) @ V

Sharding: pure data parallelism over batch — core b computes batch element b.
No collectives. Host-side prep is layout only (transpose + bf16 cast); every
FLOP of the math runs on-device.

Device algorithm (per core):
  - inputs stream in n-slices of 512 so attention on early q-blocks starts
    after ~1/4 of the DMA
  - QT/KT [64->dup 128, 2048] projections on TensorE (W chunks stationary);
    QT/KT are duplicated onto partitions 64-127 so pairs of S matmuls
    (contraction only 64-deep) run concurrently in disjoint PE row groups
  - V transposed back to natural [128, 65] tiles via PE-transpose with a ones
    column appended (row-sums of P come free as row 64 of the PV matmul)
  - per k-chunk j: S.T tile [k=128, q<=512] = KT_j.T @ QT_qb (exact-causal
    widths); exp via ACT straight PSUM->SBUF bf16 with 1/sqrt(512) folded in;
    diagonal 128x128 blocks masked by affine_select on GpSimd
  - O.T [65, q] += V'_j.T @ P.T accumulated in PSUM over k-chunks
  - epilogue per q-block: PE-transpose O.T, divide rows by the sums column,
    DMA out f32
"""

import math

import numpy as np
import ml_dtypes

import concourse.bass as bass
import concourse.tile as tile
from concourse import bacc, mybir
from concourse import masks
from concourse.bass_utils import run_bass_kernel_spmd

P = 128            # partitions / k-chunk size
N = 2048           # sequence length
D = 512            # embedding dim
DH = 64            # head dim
EC = D // P        # 4 e-chunks for the projection contraction
KC = N // P        # 16 k-chunks
QW = 512           # q block width
NQB = N // QW      # 4 q blocks / n slices
SCALE = 1.0 / math.sqrt(float(D))

BF16 = mybir.dt.bfloat16
F32 = mybir.dt.float32

_BUILD_CACHE = {}

# structural knobs (tuned via TimelineSim sweeps; defaults = best known)
OPTS = {
    "sbufs": 2,            # s psum pool buffers
    "jbufs": 2,            # proj/transpose psum pool buffers
    "dma_mode": "fine_split",  # input DMA granularity/queue split
    "mask_dve": True,      # diag causal mask: DVE tri-multiply vs gpsimd affine
    "out_batch": True,     # batch output DMA per q-block
    "order": "phases",     # attention emission order: phases | chained | trail
    "pe_warm": 20,         # dummy matmuls at t=0 to lift the HAM clock gate
}


def _ensure_ntff_hook():
    """Install the antenv.axon_hooks shim so trace=True works under axon."""
    try:
        import antenv.axon_hooks  # noqa: F401
        return
    except ImportError:
        pass
    import sys
    import types

    try:
        from trn_agent_boot.trn_boot import _ntff_profile_via_ctypes
        hook = _ntff_profile_via_ctypes("/opt/axon/libaxon_pjrt.so")
    except Exception:
        hook = None
    mod = types.ModuleType("antenv.axon_hooks")
    state = {"hook": hook}
    mod.get_axon_ntff_profile_hook = lambda: state["hook"]
    mod.set_axon_ntff_profile_hook = lambda h: state.update(hook=h)
    sys.modules["antenv.axon_hooks"] = mod
    import antenv

    antenv.axon_hooks = mod


def _build(causal: bool, has_padding: bool):
    nc = bacc.Bacc("TRN2", target_bir_lowering=False, debug=False, num_devices=8)

    xq_d = nc.dram_tensor("xq_t", [D, N], BF16, kind="ExternalInput")
    xk_d = nc.dram_tensor("xk_t", [D, N], BF16, kind="ExternalInput")
    xv_d = nc.dram_tensor("xv_t", [D, N], BF16, kind="ExternalInput")
    # w is host-duplicated [D, 2*DH] = [W | W] so the Q/K projections emit
    # [128, q] tiles whose partition halves are copies — S matmul pairs can
    # then row-pack into disjoint PE row groups with no cross-partition copy.
    w_d = nc.dram_tensor("w", [D, 2 * DH], BF16, kind="ExternalInput")
    if has_padding:
        km_d = nc.dram_tensor("kmask", [KC, P], F32, kind="ExternalInput")
    out_d = nc.dram_tensor("out", [N, DH], F32, kind="ExternalOutput")

    with tile.TileContext(nc) as tc:
        with (
            tc.tile_pool(name="const", bufs=1) as cpool,
            tc.tile_pool(name="x", bufs=12) as xpool,
            tc.tile_pool(name="big", bufs=1) as bigpool,
            tc.tile_pool(name="p", bufs=8) as ppool,
            tc.tile_pool(name="epi", bufs=2) as epipool,
            tc.tile_pool(name="o", bufs=4, space="PSUM") as opool,
            tc.tile_pool(name="s", bufs=OPTS["sbufs"], space="PSUM") as spool,
            tc.tile_pool(name="j", bufs=OPTS["jbufs"], space="PSUM") as jpool,
        ):
            # --- ACT warmup (hide exp table load behind the DMA window) ---
            warm = cpool.tile([P, 1], F32)
            nc.vector.memset(warm[:], 0.0)
            nc.scalar.activation(warm[:], warm[:], mybir.ActivationFunctionType.Exp)

            # --- PE warmup: HAM clock-gates the PE array to 1.2 GHz until it
            # sees ~3.4us of sustained matmul activity; spin dummy matmuls
            # during the DMA window so real work runs at 2.4 GHz ---
            if OPTS["pe_warm"]:
                wjunk = cpool.tile([P, P], BF16)
                nc.vector.memset(wjunk[:], 0.25)
                wpsum = opool.tile([DH + 1, QW], F32, tag="o", name="warmps")
                for _ in range(OPTS["pe_warm"]):
                    nc.tensor.matmul(
                        wpsum[:, :P], wjunk[:, :DH + 1], wjunk[:, :P],
                        start=True, stop=True, skip_group_check=True,
                    )

            ident = cpool.tile([P, P], F32)
            masks.make_identity(nc, ident[:])
            # upper-triangular (incl diag) 0/1 mask in [k, q] coords for the
            # causal diagonal blocks; multiply on DVE (gpsimd's slow semaphore
            # handling would sit in the exp->PV chain otherwise)
            tri = cpool.tile([P, P], BF16)
            masks.make_upper_triangular(nc, tri[:], val=1.0, diag=True)

            w_sb = cpool.tile([P, EC, 2 * DH], BF16)
            nc.sync.dma_start(w_sb[:], w_d.ap().rearrange("(c p) d -> p c d", p=P))
            if has_padding:
                km_sb = cpool.tile([P, KC], F32)
                nc.sync.dma_start(km_sb[:], km_d.ap().transpose([1, 0]))

            # --- input DMAs: issue spread over queues
            # (DMA issue is ~800ns serial per op on the issuing engine) ---
            x_sb = {}
            mode = OPTS["dma_mode"]
            if mode == "half":
                nch, chw = 2, 2 * QW
                engs = {"q": nc.sync, "k": nc.sync, "v": nc.gpsimd}
            elif mode == "fine_split":
                nch, chw = NQB, QW
                engs = {"q": nc.sync, "k": nc.sync, "v": nc.gpsimd}
            else:  # "fine"
                nch, chw = NQB, QW
                engs = {"q": nc.sync, "k": nc.sync, "v": nc.sync}
            for nh in range(nch):
                for tname, xd in (("q", xq_d), ("k", xk_d), ("v", xv_d)):
                    t = xpool.tile([P, EC, chw], BF16, tag="x")
                    engs[tname].dma_start(
                        t[:],
                        xd.ap()[:, nh * chw:(nh + 1) * chw].rearrange(
                            "(c p) q -> p c q", p=P
                        ),
                    )
                    x_sb[(tname, nh)] = t

            qt = bigpool.tile([P, N], BF16, tag="qt")   # rows 0-63 QT, 64-127 dup
            kt = bigpool.tile([P, N], BF16, tag="kt")
            vt = bigpool.tile([DH, N], F32, tag="vt")
            v_sb = bigpool.tile([P, KC, DH + 1], BF16, tag="vn")

            # --- projections + V-natural, per n-slice ---
            for ns in range(NQB):
                sl = slice(ns * QW, (ns + 1) * QW)
                if OPTS["dma_mode"] == "half":
                    nh, qo = ns // 2, (ns % 2) * QW
                else:
                    nh, qo = ns, 0
                for tname in ("q", "k", "v"):
                    wide = tname != "v"   # q/k project through [W|W] -> M=128
                    m = P if wide else DH
                    ps = jpool.tile([P, QW], F32, tag="j")
                    for c in range(EC):
                        nc.tensor.matmul(
                            ps[:m, :],
                            w_sb[:, c, :m],
                            x_sb[(tname, nh)][:, c, qo:qo + QW],
                            start=(c == 0),
                            stop=(c == EC - 1),
                        )
                    if tname == "q":
                        nc.vector.tensor_copy(qt[:, sl], ps[:])
                    elif tname == "k":
                        nc.vector.tensor_copy(kt[:, sl], ps[:])
                    else:
                        nc.vector.tensor_copy(vt[:, sl], ps[:DH, :])
                # V natural tiles for this n-slice: PE transpose + ones column
                vtp = jpool.tile([P, NQB, DH + 1], F32, tag="j")
                for i in range(NQB):
                    j = ns * NQB + i
                    nc.tensor.transpose(
                        vtp[:, i, :DH], vt[:, j * P:(j + 1) * P], ident[:DH, :DH]
                    )
                nc.vector.memset(vtp[:, :, DH], 1.0)
                nc.vector.tensor_copy(v_sb[:, ns * NQB:(ns + 1) * NQB, :], vtp[:])

            # --- attention: k-chunk pairs (row-packed S), q-blocks inner ---
            o_tiles = [
                opool.tile([DH + 1, QW], F32, tag="o", name=f"o{qb}")
                for qb in range(NQB)
            ]
            def emit_s(j, qb, idx, p_tiles):
                base = DH * idx
                q_off = max(0, j * P - qb * QW) if causal else 0
                width = QW - q_off
                s_ps = spool.tile([P, QW], F32, tag="s", name=f"s{j}_{qb}")
                nc.tensor.matmul(
                    s_ps[:, :width],
                    kt[base:base + DH, j * P:(j + 1) * P],
                    qt[base:base + DH, qb * QW + q_off:(qb + 1) * QW],
                    start=True,
                    stop=True,
                )
                p_sb = ppool.tile([P, QW], BF16, tag="p", name=f"p{j}_{qb}")
                nc.scalar.activation(
                    p_sb[:, :width],
                    s_ps[:, :width],
                    mybir.ActivationFunctionType.Exp,
                    scale=SCALE,
                )
                if causal and qb == j // NQB:
                    # diagonal block at cols [0,128): keep q_loc >= k_loc
                    if OPTS["mask_dve"]:
                        nc.vector.tensor_mul(p_sb[:, :P], p_sb[:, :P], tri[:])
                    else:
                        nc.gpsimd.affine_select(
                            out=p_sb[:, :P],
                            in_=p_sb[:, :P],
                            compare_op=mybir.AluOpType.is_ge,
                            fill=0.0,
                            base=0,
                            pattern=[[1, P]],
                            channel_multiplier=-1,
                        )
                if has_padding:
                    nc.vector.tensor_scalar_mul(
                        p_sb[:, :width], p_sb[:, :width], km_sb[:, j:j + 1]
                    )
                p_tiles[(j, qb)] = (p_sb, q_off, width)

            def emit_pv(j, qb, p_tiles):
                p_sb, q_off, width = p_tiles.pop((j, qb))
                j_last = ((QW // P) * (qb + 1) - 1) if causal else (KC - 1)
                nc.tensor.matmul(
                    o_tiles[qb][:, q_off:QW],
                    v_sb[:, j, :],
                    p_sb[:, :width],
                    start=(j == 0),
                    stop=(j == j_last),
                )

            for tp in range(KC // 2):
                js = (2 * tp, 2 * tp + 1)
                qb_lo = (js[0] // NQB) if causal else 0
                p_tiles = {}
                order = OPTS["order"]
                if order == "phases":
                    for qb in range(qb_lo, NQB):
                        for idx, j in enumerate(js):
                            emit_s(j, qb, idx, p_tiles)
                    for j in js:
                        for qb in range(qb_lo, NQB):
                            emit_pv(j, qb, p_tiles)
                elif order == "chained":
                    for qb in range(qb_lo, NQB):
                        for idx, j in enumerate(js):
                            emit_s(j, qb, idx, p_tiles)
                        for j in js:
                            emit_pv(j, qb, p_tiles)
                else:  # trail: PV lags S by one q-block
                    for qb in range(qb_lo, NQB):
                        for idx, j in enumerate(js):
                            emit_s(j, qb, idx, p_tiles)
                        if qb > qb_lo:
                            for j in js:
                                emit_pv(j, qb - 1, p_tiles)
                    for j in js:
                        emit_pv(j, NQB - 1, p_tiles)

                # epilogue for q-blocks completed by this pair
                done_qb = []
                if causal:
                    if js[1] % (QW // P) == QW // P - 1:
                        done_qb = [js[1] // (QW // P)]
                elif tp == KC // 2 - 1:
                    done_qb = list(range(NQB))
                for qb in done_qb:
                    oT = epipool.tile([DH + 1, QW], F32, tag="ot")
                    nc.vector.tensor_copy(oT[:], o_tiles[qb][:])
                    etp = jpool.tile([P, NQB, DH + 1], F32, tag="j")
                    for i in range(NQB):
                        nc.tensor.transpose(
                            etp[:, i, :], oT[:, i * P:(i + 1) * P],
                            ident[:DH + 1, :DH + 1],
                        )
                    recip = epipool.tile([P, NQB], F32, tag="recip")
                    nc.vector.reciprocal(recip[:], etp[:, :, DH])
                    if OPTS["out_batch"]:
                        o_sb = epipool.tile([P, NQB, DH], F32, tag="osb")
                        for i in range(NQB):
                            nc.vector.tensor_scalar_mul(
                                o_sb[:, i, :], etp[:, i, :DH], recip[:, i:i + 1]
                            )
                        nc.sync.dma_start(
                            out_d.ap()[qb * QW:(qb + 1) * QW, :].rearrange(
                                "(i p) d -> p i d", p=P
                            ),
                            o_sb[:],
                        )
                    else:
                        for i in range(NQB):
                            o_sb = epipool.tile([P, DH], F32, tag="osb")
                            nc.vector.tensor_scalar_mul(
                                o_sb[:], etp[:, i, :DH], recip[:, i:i + 1]
                            )
                            row = (qb * NQB + i) * P
                            nc.sync.dma_start(
                                out_d.ap()[row:row + P, :], o_sb[:]
                            )

    nc.compile()
    return nc


def _get(causal: bool, has_padding: bool):
    key = (causal, has_padding)
    if key not in _BUILD_CACHE:
        _BUILD_CACHE[key] = _build(causal, has_padding)
    return _BUILD_CACHE[key]


def run(key_input, query_input, value_input, padding_mask, masked_attention,
        W_key, W_query=None, W_value=None, trace=False, **_ignored):
    key_input = np.asarray(key_input, dtype=np.float32)
    query_input = np.asarray(query_input, dtype=np.float32)
    value_input = np.asarray(value_input, dtype=np.float32)
    padding_mask = np.asarray(padding_mask)
    W_key = np.asarray(W_key, dtype=np.float32)

    B = key_input.shape[0]
    causal = bool(int(np.asarray(masked_attention)))
    has_padding = bool(padding_mask.any())
    nc = _get(causal, has_padding)

    bf = ml_dtypes.bfloat16
    w_b = np.ascontiguousarray(
        np.concatenate([W_key, W_key], axis=1).astype(bf)
    )
    in_maps = []
    for b in range(B):
        m = {
            "xq_t": np.ascontiguousarray(query_input[b].T.astype(bf)),
            "xk_t": np.ascontiguousarray(key_input[b].T.astype(bf)),
            "xv_t": np.ascontiguousarray(value_input[b].T.astype(bf)),
            "w": w_b,
        }
        if has_padding:
            # multiplicative key mask in [KC, P] layout: 0 where padded
            km = (~padding_mask[b].reshape(N)).astype(np.float32)
            m["kmask"] = np.ascontiguousarray(km.reshape(KC, P))
        in_maps.append(m)

    if trace:
        _ensure_ntff_hook()
    res = run_bass_kernel_spmd(nc, in_maps, core_ids=list(range(B)), trace=trace)
    out = np.stack([np.asarray(res.results[b]["out"]) for b in range(B)], axis=0)
    return out.astype(np.float32), res


def kernel(**inputs) -> np.ndarray:
    out, _ = run(**inputs)
    return out

